# revision 1
# baseline (speedup 1.0000x reference)
"""Trainium2 Bass kernel for nn_EquivariantProductBasisWithSelfMagmomBlock.

Data-parallel over nodes: 8 NeuronCores x 8192 nodes each.

Channel-major design: per 512-node supertile, PE transposes the node-major
inputs into channel-major [c, n] tiles, all elementwise math runs on fp16
[128, 512] tiles (DVE 4x mode), matmul path weights / MLP / output linears
run as fp16 matmuls with fp32 PSUM accumulation.  The output linear uses the
channel-major mid tensors directly as matmul stationaries, producing
node-major output in PSUM (no back-transposes); a0/a1-scaled mid tensors
carry a 1/16 factor (folded into the broadcast) paired with 16x-scaled
output weights to keep fp16 products in range.

Node map inside a core: local node n = s*512 + q*128 + p.
"""

import sys

sys.path.insert(0, "/opt/trn_rl_repo")

from contextlib import ExitStack

import numpy as np

import concourse.bass as bass
import concourse.tile as tile
from concourse import bacc, mybir
from concourse.bass_utils import run_bass_kernel_spmd
from concourse.masks import make_identity

FP32 = mybir.dt.float32
F32R = mybir.dt.float32r
FP16 = mybir.dt.float16
AF = mybir.ActivationFunctionType
OP = mybir.AluOpType

N = 65536
C = 128
E = 10
INV = 16
N_CORES = 8
N_CORE = N // N_CORES  # 8192
P = 128
G = 512  # nodes per supertile

SCL = 16.0  # fp16 range guard: A-tiles carry 1/SCL, W_l* weights carry SCL


def r(ap):
    """bitcast an AP to float32r for full-rate fp32 matmul/transpose."""
    return ap.bitcast(F32R)


def build_program(n_tiles, use_silu=True):
    """Build the per-core SPMD program. n_tiles tiles of 128 nodes each.

    use_silu=False swaps Act-fused silu for sigmoid+DVE-mul (CoreSim lacks
    a Silu implementation; hardware has it in the silu_and_others table).
    """
    nc = bacc.Bacc(
        "TRN2", target_bir_lowering=False, debug=False, num_devices=N_CORES
    )
    n_nodes = n_tiles * P
    assert n_tiles % 4 == 0
    n_st = n_tiles // 4

    def din(name, shape):
        return nc.dram_tensor(name, list(shape), FP32, kind="ExternalInput").ap()

    nf_d = din("node_feats", (n_nodes, 4 * C))
    sc_d = din("sc", (n_nodes, 4 * C))
    attrs_d = din("node_attrs", (n_nodes, E))
    inv_d = din("magmom_node_inv_feats", (n_nodes, INV))
    mag_d = din("magmom_node_attrs", (n_nodes, 4))
    wsc0_d = din("w_sc0", (E, 5 * C))
    wsc1_d = din("w_sc1", (E, 4 * C))
    w1_d = din("w_mlp1", (INV, 64))
    w2_d = din("w_mlp2", (64, 64))
    w3_d = din("w_mlp3", (64, 64))
    w4_d = din("w_mlp4", (64, 4 * C))
    wl0_d = din("W_l0", (2 * C, C))
    wl1_d = din("W_l1", (2 * C, C))
    wo0_d = din("Wo0", (C, C))
    wo1_d = din("Wo1", (C, C))
    out_d = nc.dram_tensor("out", [n_nodes, 4 * C], FP32, kind="ExternalOutput").ap()
    scr_d = nc.dram_tensor("warmup_scratch", [G, 4 * C], FP32, kind="Internal").ap()
    scr_r = scr_d.rearrange("(q p) x -> p q x", p=P, q=4)

    # node n = s*512 + q*128 + p
    nf_r = nf_d.rearrange("(s q p) x -> p s q x", p=P, q=4)
    sc_r = sc_d.rearrange("(s q p) x -> p s q x", p=P, q=4)
    out_r = out_d.rearrange("(s q p) x -> p s q x", p=P, q=4)
    inv_r = inv_d.rearrange("(s q p) x -> p s q x", p=P, q=4)
    attrs_r = attrs_d.rearrange("(s q p) x -> p s q x", p=P, q=4)
    mag_r = mag_d.rearrange("(s q p) x -> p s q x", p=P, q=4)

    with tile.TileContext(nc) as tc, ExitStack() as ctx:
        singles = ctx.enter_context(tc.tile_pool(name="singles", bufs=1))
        nat = ctx.enter_context(tc.tile_pool(name="nat", bufs=2))
        ew = ctx.enter_context(tc.tile_pool(name="ew", bufs=2))
        # PSUM pools (8 banks): xps 1 + wz (zs 2 + zb 1) + work 2 + out 2 = 8
        xps_pool = ctx.enter_context(tc.tile_pool(name="xps", bufs=1, space="PSUM"))
        wz_pool = ctx.enter_context(tc.tile_pool(name="wzp", bufs=2, space="PSUM"))
        work_pool = ctx.enter_context(tc.tile_pool(name="work", bufs=2, space="PSUM"))
        out_pool = ctx.enter_context(tc.tile_pool(name="outp", bufs=2, space="PSUM"))

        # ---------------- preloads ----------------
        # identity is produced by gpsimd (Q7); launder it through a DVE copy
        # so PE never consumes a Q7-written tensor (HW flush hazard on the
        # very first transposes).
        ident_g = singles.tile([P, P], FP32)
        make_identity(nc, ident_g[:])
        ident = singles.tile([P, P], FP32)

        attrs_all = singles.tile([P, n_st, 4, E], FP32)
        nc.sync.dma_start(out=attrs_all[:], in_=attrs_r)
        inv_all = singles.tile([P, n_st, 4, INV], FP32)
        nc.sync.dma_start(out=inv_all[:], in_=inv_r)
        mag_all = singles.tile([P, n_st, 4, 4], FP32)
        nc.sync.dma_start(out=mag_all[:], in_=mag_r)

        wscf = singles.tile([E, 9 * C], FP32)
        nc.sync.dma_start(out=wscf[:, 0 : 5 * C], in_=wsc0_d)
        nc.sync.dma_start(out=wscf[:, 5 * C : 9 * C], in_=wsc1_d)
        wsc_h = singles.tile([E, 9 * C], FP16)
        nc.vector.tensor_copy(wsc_h[:], wscf[:])

        w1f = singles.tile([INV, 64], FP32)
        nc.sync.dma_start(out=w1f[:], in_=w1_d)
        w2f = singles.tile([64, 64], FP32)
        nc.sync.dma_start(out=w2f[:], in_=w2_d)
        w3f = singles.tile([64, 64], FP32)
        nc.sync.dma_start(out=w3f[:], in_=w3_d)
        w4f = singles.tile([64, 4 * C], FP32)
        nc.sync.dma_start(out=w4f[:], in_=w4_d)
        w1h = singles.tile([INV, 64], FP16)
        nc.vector.tensor_copy(w1h[:], w1f[:])
        w2h = singles.tile([64, 64], FP16)
        nc.vector.tensor_copy(w2h[:], w2f[:])
        w3h = singles.tile([64, 64], FP16)
        nc.vector.tensor_copy(w3h[:], w3f[:])
        w4h = singles.tile([64, 4 * C], FP16)
        nc.vector.tensor_copy(w4h[:], w4f[:])
        # laundering copy sits late in the in-order DVE queue: by the time it
        # dispatches, the Q7 identity write has long since landed.
        nc.vector.tensor_copy(ident[:], ident_g[:])

        # output weights: 0=WA0*S 1=WB0*S 2=WA1*S 3=WB1*S 4=Wo0 5=Wo1
        Wf = singles.tile([P, 6, C], FP32)
        nc.sync.dma_start(out=Wf[:, 0, :], in_=wl0_d[0:128, :])
        nc.sync.dma_start(out=Wf[:, 1, :], in_=wl0_d[128:256, :])
        nc.sync.dma_start(out=Wf[:, 2, :], in_=wl1_d[0:128, :])
        nc.sync.dma_start(out=Wf[:, 3, :], in_=wl1_d[128:256, :])
        nc.sync.dma_start(out=Wf[:, 4, :], in_=wo0_d)
        nc.sync.dma_start(out=Wf[:, 5, :], in_=wo1_d)
        Wh = singles.tile([P, 6, C], FP16)
        nc.scalar.activation(Wh[:, 0:4, :], Wf[:, 0:4, :], AF.Copy, scale=SCL)
        nc.scalar.copy(Wh[:, 4:6, :], Wf[:, 4:6, :])

        ones_t = singles.tile([1, P], FP16)
        nc.vector.memset(ones_t[:], 1.0 / SCL)

        def emit(s_, warmup=False):
            sl = slice(s_ * G, (s_ + 1) * G)
            # ---------------- supertile loads ----------------
            nf_st = nat.tile([P, 16 * C], FP32, tag="nf")
            nc.sync.dma_start(out=nf_st[:].rearrange("p (q x) -> p q x", q=4), in_=nf_r[:, s_])
            sc_st = nat.tile([P, 16 * C], FP32, tag="sc")
            nc.sync.dma_start(out=sc_st[:].rearrange("p (q x) -> p q x", q=4), in_=sc_r[:, s_])
            out_st = nat.tile([P, 16 * C], FP32, tag="out")

            nfv = nf_st[:].rearrange("p (q c j) -> p q c j", q=4, j=4)

            # ------- attrs / inv / a1x transposes (partition-0 psum tiles) -------
            smA = work_pool.tile([E, G], FP32, tag="w")
            smI = work_pool.tile([INV, G], FP32, tag="w")
            smM = [work_pool.tile([1, G], FP32, tag="w", name=f"smM{i}") for i in range(4)]
            for q in range(4):
                qs = slice(q * P, (q + 1) * P)
                nc.tensor.transpose(smA[:, qs], attrs_all[:, s_, q, :], ident[:])
                nc.tensor.transpose(smI[:, qs], inv_all[:, s_, q, :], ident[:])
                for i in range(4):
                    nc.tensor.transpose(
                        smM[i][:, qs], mag_all[:, s_, q, i : i + 1], ident[:]
                    )
            aT = ew.tile([E, G], FP16, tag="aT")
            nc.vector.tensor_copy(aT[:], smA[:])
            iT = ew.tile([INV, G], FP16, tag="iT")
            nc.vector.tensor_copy(iT[:], smI[:])
            magh = ew.tile([1, 4, G], FP16, tag="magh")
            for i in range(4):
                nc.vector.tensor_copy(magh[:, i, :], smM[i][:])

            # ------- x transposes to channel-major + fp16 copies (Act) -------
            x0f = ew.tile([P, G], FP32, tag="x0f")
            xh = ew.tile([P, 3, G], FP16, tag="xh")
            for comp in range(4):
                xp = xps_pool.tile([P, G], FP32, tag="x")
                for q in range(4):
                    nc.tensor.transpose(
                        xp[:, q * P : (q + 1) * P], nfv[:, q, :, comp], ident[:]
                    )
                if comp == 0:
                    nc.scalar.copy(x0f[:], xp[:])
                else:
                    nc.scalar.copy(xh[:, comp - 1, :], xp[:])
            x0 = x0f[:]
            x1sl = xh[:, 0:3, :]

            # squares: x0^2 fp32 on Act, x1 squares fp16 ; n1 = |x1|^2 (fp32)
            sq0t = ew.tile([P, G], FP32, tag="sq0t")
            nc.scalar.activation(sq0t[:], x0f[:], AF.Square)
            sq0 = sq0t[:]
            sq1 = ew.tile([P, 3, G], FP16, tag="sq1", bufs=1)
            nc.scalar.activation(sq1[:], xh[:], AF.Square)
            n1t = ew.tile([P, G], FP32, tag="n1")
            nc.vector.tensor_add(n1t[:], sq1[:, 0, :], sq1[:, 1, :])
            nc.vector.tensor_add(n1t[:], n1t[:], sq1[:, 2, :])

            # ------- A broadcasts (PE ones-matmul, carries 1/SCL; Act copies) ----
            A1 = ew.tile([P, 3, G], FP16, tag="A1")
            for m in range(3):
                bp = work_pool.tile([P, G], FP32, tag="w")
                nc.tensor.matmul(bp[:], ones_t[0:1, :], magh[:, 1 + m, :])
                nc.scalar.copy(A1[:, m, :], bp[:])
            A0h = ew.tile([64, G], FP16, tag="A0h")
            bp = work_pool.tile([P, G], FP32, tag="w")
            nc.tensor.matmul(bp[0:64, :], ones_t[0:1, 0:64], magh[:, 0, :])
            nc.scalar.copy(A0h[:], bp[0:64, :])

            # ------- wz paths; bases accumulate in PSUM via start=False mm -------
            def wz_mm(k, out=None, start=True, stop=True):
                if out is None:
                    out = wz_pool.tile([P, G], FP32, tag="zs")
                nc.tensor.matmul(
                    out[:], wsc_h[:, k * P : (k + 1) * P], aT[:],
                    start=start, stop=stop, skip_group_check=True,
                )
                return out

            # a = wz0 + wz1*x0 + wz3*sq0  (B accumulates wz1*x0 then +wz0)
            wp = wz_mm(1)
            Bb = wz_pool.tile([P, G], FP32, tag="zb", bufs=1)
            nc.vector.tensor_mul(Bb[:], wp[:], x0)
            wz_mm(0, out=Bb, start=False, stop=True)
            wp = wz_mm(3)
            t2 = ew.tile([P, G], FP32, tag="t2", bufs=1)
            nc.vector.tensor_mul(t2[:], wp[:], sq0)
            av = ew.tile([P, G], FP32, tag="av", bufs=1)
            nc.vector.tensor_add(av[:], Bb[:], t2[:])
            # c1 = wz5 + wz6*x0 + wz7*sq0 + wz8*n1
            wp = wz_mm(6)
            m1 = ew.tile([P, G], FP32, tag="m1", bufs=1)
            nc.vector.tensor_mul(m1[:], wp[:], x0)
            wp = wz_mm(5)
            c1 = ew.tile([P, G], FP32, tag="c1")
            nc.vector.tensor_add(c1[:], wp[:], m1[:])
            wp = wz_mm(7)
            m2 = ew.tile([P, G], FP32, tag="m2", bufs=1)
            nc.vector.tensor_mul(m2[:], wp[:], sq0)
            nc.vector.tensor_add(c1[:], c1[:], m2[:])
            wp = wz_mm(8)
            m3 = ew.tile([P, G], FP32, tag="m3", bufs=1)
            nc.vector.tensor_mul(m3[:], wp[:], n1t[:])
            nc.vector.tensor_add(c1[:], c1[:], m3[:])
            # b = wz2 + wz4*x0 ; y0 = x0*a + n1*b
            wp = wz_mm(4)
            t4 = ew.tile([P, G], FP32, tag="t4", bufs=1)
            nc.vector.tensor_mul(t4[:], wp[:], x0)
            wp = wz_mm(2)
            bv = ew.tile([P, G], FP32, tag="bv", bufs=1)
            nc.vector.tensor_add(bv[:], wp[:], t4[:])
            y0 = ew.tile([P, G], FP32, tag="y0")
            ya = ew.tile([P, G], FP32, tag="ya", bufs=1)
            nc.vector.tensor_mul(ya[:], x0, av[:])
            nc.vector.tensor_mul(y0[:], n1t[:], bv[:])
            nc.vector.tensor_add(y0[:], y0[:], ya[:])

            y0h = ew.tile([P, G], FP16, tag="y0h")
            nc.vector.tensor_copy(y0h[:], y0[:])
            c1h = ew.tile([P, G], FP16, tag="c1h")
            nc.vector.tensor_copy(c1h[:], c1[:])

            # y1m = c1 * x1m  (Pool, batched via stride-0 rep of c1)
            y1t = ew.tile([P, 3, G], FP16, tag="y1t")
            c1ap = c1h[:]
            c1b = bass.AP(
                tensor=c1ap.tensor, offset=c1ap.offset,
                ap=[c1ap.ap[0], [0, 3], c1ap.ap[1]],
            )
            nc.vector.tensor_mul(y1t[:], c1b, x1sl)

            # s = sum_m y1m * A1m  (carries 1/SCL; Pool mul, DVE adds)
            smul = ew.tile([P, 3, G], FP16, tag="smul", bufs=1)
            nc.vector.tensor_mul(smul[:], y1t[:], A1[:])
            sv = ew.tile([P, G], FP16, tag="sv")
            nc.vector.tensor_add(sv[:], smul[:, 0, :], smul[:, 1, :])
            nc.vector.tensor_add(sv[:], sv[:], smul[:, 2, :])

            # ------- magmom MLP (channel-major) -------
            h = iT
            hw_ = [w1h, w2h, w3h]
            for li in range(3):
                hp = work_pool.tile([64, G], FP32, tag="w")
                nc.tensor.matmul(hp[:], hw_[li][:], h[:])
                hn = ew.tile([64, G], FP16, tag=f"h{li}")
                sg = ew.tile([64, G], FP16, tag=f"sg{li}")
                nc.scalar.activation(sg[:], hp[:], AF.Sigmoid)
                nc.vector.tensor_mul(hn[:], hp[:], sg[:])
                h = hn
            # a0-scaled copy of h3 feeds the wa/wd matmuls (folds a0/SCL in)
            h3a = ew.tile([64, G], FP16, tag="h3a")
            nc.vector.tensor_mul(h3a[:], h[:], A0h[:])

            # tpw quarters: wa,wd use h3a (a0-scaled); wb,wc use h
            wp = work_pool.tile([P, G], FP32, tag="w")
            nc.tensor.matmul(wp[:], w4h[:, 0:P], h3a[:])
            mid0a = ew.tile([P, G], FP16, tag="mid0a")
            nc.vector.tensor_mul(mid0a[:], wp[:], y0[:])
            wp = work_pool.tile([P, G], FP32, tag="w")
            nc.tensor.matmul(wp[:], w4h[:, P : 2 * P], h[:])
            g2 = ew.tile([P, G], FP16, tag="g2")
            nc.vector.tensor_mul(g2[:], wp[:], sv[:])
            wp = work_pool.tile([P, G], FP32, tag="w")
            nc.tensor.matmul(wp[:], w4h[:, 2 * P : 3 * P], h[:])
            wcy0 = ew.tile([P, G], FP16, tag="wcy0")
            nc.vector.tensor_mul(wcy0[:], wp[:], y0[:])
            m1c = ew.tile([P, 3, G], FP16, tag="m1c", bufs=1)
            wcap = wcy0[:]
            wcb = bass.AP(
                tensor=wcap.tensor, offset=wcap.offset,
                ap=[wcap.ap[0], [0, 3], wcap.ap[1]],
            )
            nc.vector.tensor_mul(m1c[:], wcb, A1[:])
            wp = work_pool.tile([P, G], FP32, tag="w")
            nc.tensor.matmul(wp[:], w4h[:, 3 * P : 4 * P], h3a[:])
            rc2 = ew.tile([P, G], FP16, tag="rc2")
            nc.vector.tensor_mul(rc2[:], wp[:], c1[:])
            hm = ew.tile([P, 3, G], FP16, tag="hm", bufs=1)
            rcap = rc2[:]
            rcb = bass.AP(
                tensor=rcap.tensor, offset=rcap.offset,
                ap=[rcap.ap[0], [0, 3], rcap.ap[1]],
            )
            nc.vector.tensor_mul(hm[:], rcb, x1sl)

            # ------- output linears: node-major PSUM via mid-stationary -------
            outv = out_st[:].rearrange("p (q f) -> p q f", q=4)
            scv = sc_st[:].rearrange("p (q f) -> p q f", q=4)

            o0p = out_pool.tile([P, 4, P], FP32, tag="o")
            for q in range(4):
                qs = slice(q * P, (q + 1) * P)
                nc.tensor.matmul(o0p[:, q, :], mid0a[:, qs], Wh[:, 0, :], start=True, stop=False)
                nc.tensor.matmul(o0p[:, q, :], g2[:, qs], Wh[:, 1, :], start=False, stop=False)
                nc.tensor.matmul(o0p[:, q, :], y0h[:, qs], Wh[:, 4, :], start=False, stop=True)
            nc.vector.tensor_add(outv[:, :, 0:C], o0p[:], scv[:, :, 0:C])

            for m in range(3):
                o1p = out_pool.tile([P, 4, P], FP32, tag="o")
                for q in range(4):
                    qs = slice(q * P, (q + 1) * P)
                    nc.tensor.matmul(o1p[:, q, :], m1c[:, m, qs], Wh[:, 2, :], start=True, stop=False)
                    nc.tensor.matmul(o1p[:, q, :], hm[:, m, qs], Wh[:, 3, :], start=False, stop=False)
                    nc.tensor.matmul(o1p[:, q, :], y1t[:, m, qs], Wh[:, 5, :], start=False, stop=True)
                ovm = outv[:, :, C : 4 * C].rearrange("p q (c j) -> p q c j", j=3)[:, :, :, m]
                svm = scv[:, :, C : 4 * C].rearrange("p q (c j) -> p q c j", j=3)[:, :, :, m]
                nc.vector.tensor_add(ovm, o1p[:], svm)

            tgt = scr_r if warmup else out_r[:, s_]
            nc.sync.dma_start(out=tgt, in_=out_st[:].rearrange("p (q x) -> p q x", q=4))


        # sacrificial first supertile: absorbs any cold-start races (first
        # ldweights/reads of engine-produced preloads); result discarded.
        emit(0, warmup=True)
        for s_ in range(n_st):
            emit(s_)

    nc.compile()
    return nc


_CACHE = {}


def _get_program(n_tiles):
    if n_tiles not in _CACHE:
        import os
        _CACHE[n_tiles] = build_program(
            n_tiles, use_silu=os.environ.get("K_NO_SILU", "") != "1"
        )
    return _CACHE[n_tiles]


def _in_map_for_core(inputs, c, n_core):
    lo, hi = c * n_core, (c + 1) * n_core
    return {
        "node_feats": np.ascontiguousarray(
            inputs["node_feats"][lo:hi].reshape(n_core, 4 * C)
        ),
        "sc": np.ascontiguousarray(inputs["sc"][lo:hi]),
        "node_attrs": np.ascontiguousarray(inputs["node_attrs"][lo:hi]),
        "magmom_node_inv_feats": np.ascontiguousarray(
            inputs["magmom_node_inv_feats"][lo:hi]
        ),
        "magmom_node_attrs": np.ascontiguousarray(inputs["magmom_node_attrs"][lo:hi]),
        "w_sc0": np.ascontiguousarray(inputs["w_sc0"].reshape(E, 5 * C)),
        "w_sc1": np.ascontiguousarray(inputs["w_sc1"].reshape(E, 4 * C)),
        "w_mlp1": np.asarray(inputs["w_mlp1"]),
        "w_mlp2": np.asarray(inputs["w_mlp2"]),
        "w_mlp3": np.asarray(inputs["w_mlp3"]),
        "w_mlp4": np.asarray(inputs["w_mlp4"]),
        "W_l0": np.asarray(inputs["W_l0"]),
        "W_l1": np.asarray(inputs["W_l1"]),
        "Wo0": np.asarray(inputs["Wo0"]),
        "Wo1": np.asarray(inputs["Wo1"]),
    }


def run_on_hw(inputs, trace=False):
    inputs = {k: np.asarray(v, dtype=np.float32) for k, v in inputs.items()}
    n_nodes = inputs["node_feats"].shape[0]
    n_core = n_nodes // N_CORES
    nc = _get_program(n_core // P)
    in_maps = [_in_map_for_core(inputs, c, n_core) for c in range(N_CORES)]
    res = run_bass_kernel_spmd(
        nc, in_maps, core_ids=list(range(N_CORES)), trace=trace
    )
    out = np.concatenate([res.results[c]["out"] for c in range(N_CORES)], axis=0)
    return out.astype(np.float32), res


def kernel(**inputs) -> np.ndarray:
    import os, time

    os.environ.setdefault("NEURON_RT_RESET_CORES", "1")
    try:
        out, _ = run_on_hw(inputs, trace=False)
    except Exception:
        time.sleep(5)
        out, _ = run_on_hw(inputs, trace=False)
    return out


def bench(inputs, iters=5):
    """Pipelined timing of the sharded NEFF execution (device-resident inputs)."""
    import time
    import jax
    from jax.sharding import Mesh, PartitionSpec
    from jax.experimental.shard_map import shard_map
    from concourse import bass2jax
    from concourse.bass2jax import _bass_exec_p, install_neuronx_cc_hook

    inputs = {k: np.asarray(v, dtype=np.float32) for k, v in inputs.items()}
    n_nodes = inputs["node_feats"].shape[0]
    n_core = n_nodes // N_CORES
    nc = _get_program(n_core // P)
    in_maps = [_in_map_for_core(inputs, c, n_core) for c in range(N_CORES)]

    install_neuronx_cc_hook()
    partition_name = nc.partition_id_tensor.name if nc.partition_id_tensor else None
    in_names, out_names, out_avals, zero_outs = [], [], [], []
    for alloc in nc.m.functions[0].allocations:
        if not isinstance(alloc, mybir.MemoryLocationSet):
            continue
        name = alloc.memorylocations[0].name
        if alloc.kind == "ExternalInput":
            if name != partition_name:
                in_names.append(name)
        elif alloc.kind == "ExternalOutput":
            out_names.append(name)
            shape = tuple(alloc.tensor_shape)
            dtype = mybir.dt.np(alloc.dtype)
            out_avals.append(jax.core.ShapedArray(shape, dtype))
            zero_outs.append(np.zeros(shape, dtype))
    n_params = len(in_names)
    all_names = in_names + out_names
    if partition_name is not None:
        all_names.append(partition_name)

    def _body(*args):
        operands = list(args)
        if partition_name is not None:
            operands.append(bass2jax.partition_id_tensor())
        return tuple(
            _bass_exec_p.bind(
                *operands,
                out_avals=tuple(out_avals),
                in_names=tuple(all_names),
                out_names=tuple(out_names),
                lowering_input_output_aliases=(),
                sim_require_finite=True,
                sim_require_nnan=True,
                nc=nc,
            )
        )

    devices = jax.devices()[:N_CORES]
    mesh = Mesh(np.asarray(devices), ("core",))
    nin = n_params + len(out_names)
    sharded = jax.jit(
        shard_map(
            _body,
            mesh=mesh,
            in_specs=(PartitionSpec("core"),) * nin,
            out_specs=(PartitionSpec("core"),) * len(out_names),
            check_rep=False,
        ),
        keep_unused=True,
    )
    per_core = [[np.asarray(m[nm]) for nm in in_names] for m in in_maps]
    concat_in = [
        np.concatenate([per_core[c][i] for c in range(N_CORES)], axis=0)
        for i in range(n_params)
    ]
    concat_zeros = [
        np.zeros((N_CORES * z.shape[0], *z.shape[1:]), z.dtype) for z in zero_outs
    ]
    from jax.sharding import NamedSharding
    sh = NamedSharding(mesh, PartitionSpec("core"))
    dev_in = [jax.device_put(a, sh) for a in concat_in + concat_zeros]
    out = sharded(*dev_in)
    jax.block_until_ready(out)
    t0 = time.time()
    for _ in range(iters):
        out = sharded(*dev_in)
    jax.block_until_ready(out)
    dt = (time.time() - t0) / iters
    return dt * 1e9, out



# revision 8
# speedup vs baseline: 23.6634x; 23.6634x over previous
"""Trainium2 Bass kernel for nn_EquivariantProductBasisWithSelfMagmomBlock.

Data-parallel over nodes: 8 NeuronCores x 8192 nodes each.

Channel-major design: per 512-node supertile, PE transposes the node-major
inputs into channel-major [c, n] tiles; elementwise math runs mostly on fp16
[128, 512] tiles; matmuls run fp16 with fp32 PSUM accumulation.

v2 changes vs baseline:
 - attrs/inv/mag concatenated into one [128, 30] tile -> 4 input transposes
   per supertile instead of 24.
 - x1 components transpose into one 3-bank PSUM tile; a single Act copy
   moves all three planes to fp16 SBUF.
 - Act Silu directly from PSUM (replaces sigmoid + DVE mul per MLP layer).
 - wz chain restructured: DVE writes x0*wz products straight into PSUM and
   the companion wz term accumulates on top via a start=False matmul.
 - several SBUF-only fp16 adds/muls offloaded to the idle GpSimd engine.

PSUM budget (8 banks): x1p 3 + zs ring 2 + zb 1 + out ring 2.

Node map inside a core: local node n = s*512 + q*128 + p.
"""

import sys

sys.path.insert(0, "/opt/trn_rl_repo")

from contextlib import ExitStack

import numpy as np

import concourse.bass as bass
import concourse.tile as tile
from concourse import bacc, mybir
from concourse.bass_utils import run_bass_kernel_spmd
from concourse.masks import make_identity

FP32 = mybir.dt.float32
F32R = mybir.dt.float32r
FP16 = mybir.dt.float16
AF = mybir.ActivationFunctionType
OP = mybir.AluOpType

N = 65536
C = 128
E = 10
INV = 16
N_CORES = 8
N_CORE = N // N_CORES  # 8192
P = 128
G = 512  # nodes per supertile
CMB = 80  # padded: attrs@0:10, mag@32:36, inv@64:80 (matmul base-partition rule)

SCL = 16.0  # fp16 range guard: A-tiles carry 1/SCL, W_l* weights carry SCL


def bcast3(ap_2d):
    """[p, n] AP -> [p, 3, n] stride-0 broadcast AP on the middle dim."""
    return bass.AP(
        tensor=ap_2d.tensor, offset=ap_2d.offset,
        ap=[ap_2d.ap[0], [0, 3], ap_2d.ap[1]],
    )


def build_program(n_tiles):
    """Build the per-core SPMD program. n_tiles tiles of 128 nodes each."""
    nc = bacc.Bacc(
        "TRN2", target_bir_lowering=False, debug=False, num_devices=N_CORES
    )
    n_nodes = n_tiles * P
    assert n_tiles % 4 == 0
    n_st = n_tiles // 4

    def din(name, shape):
        return nc.dram_tensor(name, list(shape), FP32, kind="ExternalInput").ap()

    nf_d = din("node_feats", (n_nodes, 4 * C))
    sc_d = din("sc", (n_nodes, 4 * C))
    attrs_d = din("node_attrs", (n_nodes, E))
    inv_d = din("magmom_node_inv_feats", (n_nodes, INV))
    mag_d = din("magmom_node_attrs", (n_nodes, 4))
    wsc0_d = din("w_sc0", (E, 5 * C))
    wsc1_d = din("w_sc1", (E, 4 * C))
    w1_d = din("w_mlp1", (INV, 64))
    w2_d = din("w_mlp2", (64, 64))
    w3_d = din("w_mlp3", (64, 64))
    w4_d = din("w_mlp4", (64, 4 * C))
    wl0_d = din("W_l0", (2 * C, C))
    wl1_d = din("W_l1", (2 * C, C))
    wo0_d = din("Wo0", (C, C))
    wo1_d = din("Wo1", (C, C))
    out_d = nc.dram_tensor("out", [n_nodes, 4 * C], FP32, kind="ExternalOutput").ap()
    scr_d = nc.dram_tensor("warmup_scratch", [G, 4 * C], FP32, kind="Internal").ap()
    scr_r = scr_d.rearrange("(q p) x -> p q x", p=P, q=4)

    # node n = s*512 + q*128 + p
    nf_r = nf_d.rearrange("(s q p) x -> p s q x", p=P, q=4)
    sc_r = sc_d.rearrange("(s q p) x -> p s q x", p=P, q=4)
    out_r = out_d.rearrange("(s q p) x -> p s q x", p=P, q=4)
    attrs_r = attrs_d.rearrange("(s q p) x -> p s q x", p=P, q=4)
    inv_r = inv_d.rearrange("(s q p) x -> p s q x", p=P, q=4)
    mag_r = mag_d.rearrange("(s q p) x -> p s q x", p=P, q=4)

    with tile.TileContext(nc) as tc, ExitStack() as ctx:
        singles = ctx.enter_context(tc.tile_pool(name="singles", bufs=1))
        nat = ctx.enter_context(tc.tile_pool(name="nat", bufs=2))
        ew = ctx.enter_context(tc.tile_pool(name="ew", bufs=2))
        # PSUM pools (8 banks): x1p 3 + zs 2 + zb 1 + out 2
        xps_pool = ctx.enter_context(tc.tile_pool(name="xps", bufs=1, space="PSUM"))
        work_pool = ctx.enter_context(tc.tile_pool(name="work", bufs=2, space="PSUM"))
        acc_pool = ctx.enter_context(tc.tile_pool(name="accp", bufs=1, space="PSUM"))
        out_pool = ctx.enter_context(tc.tile_pool(name="outp", bufs=2, space="PSUM"))

        # ---------------- preloads ----------------
        # identity is produced by gpsimd (Q7); launder it through a DVE copy
        # so PE never consumes a Q7-written tensor.
        ident_g = singles.tile([P, P], FP32)
        make_identity(nc, ident_g[:])
        ident = singles.tile([P, P], FP32)

        # combined attrs|mag|inv per-node table, fp32; slice bases chosen so
        # each transposed block lands at a legal matmul base partition.
        cmb_all = singles.tile([P, n_st, 4, CMB], FP32)
        nc.sync.dma_start(out=cmb_all[:, :, :, 0:E], in_=attrs_r)
        nc.sync.dma_start(out=cmb_all[:, :, :, 32:36], in_=mag_r)
        nc.sync.dma_start(out=cmb_all[:, :, :, 64:64 + INV], in_=inv_r)

        wscf = singles.tile([E, 9 * C], FP32)
        nc.sync.dma_start(out=wscf[:, 0:5 * C], in_=wsc0_d)
        nc.sync.dma_start(out=wscf[:, 5 * C:9 * C], in_=wsc1_d)
        wsc_h = singles.tile([E, 9 * C], FP16)
        nc.vector.tensor_copy(wsc_h[:], wscf[:])

        w1f = singles.tile([INV, 64], FP32)
        nc.sync.dma_start(out=w1f[:], in_=w1_d)
        w2f = singles.tile([64, 64], FP32)
        nc.sync.dma_start(out=w2f[:], in_=w2_d)
        w3f = singles.tile([64, 64], FP32)
        nc.sync.dma_start(out=w3f[:], in_=w3_d)
        w4f = singles.tile([64, 4 * C], FP32)
        nc.sync.dma_start(out=w4f[:], in_=w4_d)
        w2h = singles.tile([64, 64], FP16)
        nc.vector.tensor_copy(w2h[:], w2f[:])
        w3h = singles.tile([64, 64], FP16)
        nc.vector.tensor_copy(w3h[:], w3f[:])
        w4h = singles.tile([64, 4 * C], FP16)
        nc.vector.tensor_copy(w4h[:], w4f[:])
        # laundering copy sits late in the in-order DVE queue
        nc.vector.tensor_copy(ident[:], ident_g[:])

        # output weights: 0=WA0*S 1=WB0*S 2=WA1*S 3=WB1*S 4=Wo0 5=Wo1
        Wf = singles.tile([P, 6, C], FP32)
        nc.sync.dma_start(out=Wf[:, 0, :], in_=wl0_d[0:128, :])
        nc.sync.dma_start(out=Wf[:, 1, :], in_=wl0_d[128:256, :])
        nc.sync.dma_start(out=Wf[:, 2, :], in_=wl1_d[0:128, :])
        nc.sync.dma_start(out=Wf[:, 3, :], in_=wl1_d[128:256, :])
        nc.sync.dma_start(out=Wf[:, 4, :], in_=wo0_d)
        nc.sync.dma_start(out=Wf[:, 5, :], in_=wo1_d)
        Wh = singles.tile([P, 6, C], FP16)
        nc.scalar.activation(Wh[:, 0:4, :], Wf[:, 0:4, :], AF.Copy, scale=SCL)
        nc.scalar.copy(Wh[:, 4:6, :], Wf[:, 4:6, :])

        # broadcast stationaries at base 32 (match magh rows): sel[k] picks
        # mag row 32+k and replicates it over all output partitions.
        sel32 = singles.tile([36, 4, P], FP16)
        ones36 = singles.tile([36, P], FP16)
        nc.vector.memset(ones36[:], 1.0 / SCL)
        # plane m selects mag row 32+m: sel[32+k, m, :] = (1/SCL)*delta(k==m),
        # built as ones * per-partition column e_m taken from the identity.
        for m in range(4):
            nc.vector.tensor_scalar_mul(
                sel32[32:36, m, :], ones36[32:36, :], ident_g[32:36, 32 + m:33 + m]
            )
        # MLP layer-1 stationary replica at base 64 (matches iT rows)
        w1h_rep = singles.tile([64 + INV, 64], FP16)
        nc.vector.tensor_copy(w1h_rep[64:64 + INV, :], w1f[:])

        def emit(s_, warmup=False):
            # ---------------- supertile loads ----------------
            nf_st = nat.tile([P, 16 * C], FP32, tag="nf")
            nc.sync.dma_start(
                out=nf_st[:].rearrange("p (q x) -> p q x", q=4), in_=nf_r[:, s_]
            )
            sc_st = nat.tile([P, 16 * C], FP32, tag="sc")
            nc.sync.dma_start(
                out=sc_st[:].rearrange("p (q x) -> p q x", q=4), in_=sc_r[:, s_]
            )
            out_st = nat.tile([P, 16 * C], FP32, tag="out")

            nfv = nf_st[:].rearrange("p (q c j) -> p q c j", q=4, j=4)

            zs_n = [0]

            def zs(tag="zs"):
                zs_n[0] += 1
                return work_pool.tile([P, G], FP32, tag=tag, name=f"zs{zs_n[0]}")

            # ------- combined attrs|inv|mag transpose: 4 PE ops -------
            cmbp = zs()
            for q in range(4):
                nc.tensor.transpose(
                    cmbp[0:CMB, q * P:(q + 1) * P], cmb_all[:, s_, q, :], ident[:]
                )
            cmbh = ew.tile([CMB, G], FP16, tag="cmbh")
            nc.vector.tensor_copy(cmbh[:], cmbp[0:CMB, :])
            aT = cmbh[0:E, :]
            magh = cmbh[32:36, :]  # rows: a0, a1x, a1y, a1z (base 32)
            # (consumed as matmul moving at base 32 with sel32 stationaries)
            iT = cmbh[64:64 + INV, :]  # base 64

            # ------- x transposes -> PSUM; copies to fp16 SBUF -------
            x0p = zs()
            for q in range(4):
                nc.tensor.transpose(
                    x0p[:, q * P:(q + 1) * P], nfv[:, q, :, 0], ident[:]
                )
            x0h = ew.tile([P, G], FP16, tag="x0h")
            nc.vector.tensor_copy(x0h[:], x0p[:])
            x1p = xps_pool.tile([P, 3, G], FP32, tag="x1p")
            for m in range(3):
                for q in range(4):
                    nc.tensor.transpose(
                        x1p[:, m, q * P:(q + 1) * P], nfv[:, q, :, 1 + m], ident[:]
                    )
            xh = ew.tile([P, 3, G], FP16, tag="xh")
            nc.scalar.copy(xh[:], x1p[:])

            # ------- A broadcasts (PE ones-matmul, carries 1/SCL) -------
            A1 = ew.tile([P, 3, G], FP16, tag="A1")
            for m in range(3):
                bp = zs()
                nc.tensor.matmul(bp[:], sel32[32:36, 1 + m, :], magh[:])
                nc.scalar.copy(A1[:, m, :], bp[:])
            bp = zs()
            nc.tensor.matmul(bp[0:64, :], sel32[32:36, 0, 0:64], magh[:])
            A0h = ew.tile([64, G], FP16, tag="A0h")
            nc.scalar.copy(A0h[:], bp[0:64, :])

            # ------- squares -------
            sq0 = ew.tile([P, G], FP16, tag="sq0")
            nc.vector.tensor_mul(sq0[:], x0h[:], x0h[:])
            sq1 = ew.tile([P, 3, G], FP16, tag="sq1", bufs=1)
            nc.scalar.activation(sq1[:], xh[:], AF.Square)
            n1h = ew.tile([P, G], FP16, tag="n1")
            nc.gpsimd.tensor_add(n1h[:], sq1[:, 0, :], sq1[:, 1, :])
            nc.gpsimd.tensor_add(n1h[:], n1h[:], sq1[:, 2, :])

            # ------- wz chain -------
            # A = wz0 + x0*wz1 + sq0*wz3 ; B = wz2 + x0*wz4
            # c1 = wz5 + x0*wz6 + sq0*wz7 + n1*wz8 ; y0 = x0*A + n1*B
            def wz_mm(k, out=None, start=True, stop=True):
                if out is None:
                    out = zs()
                nc.tensor.matmul(
                    out[:], wsc_h[:, k * P:(k + 1) * P], aT,
                    start=start, stop=stop, skip_group_check=True,
                )
                return out

            # A-block: AB(psum) = x0*wz1, += wz0 (PE), Av = AB + sq0*wz3
            wp = wz_mm(1)
            AB = acc_pool.tile([P, G], FP32, tag="zb")
            nc.vector.tensor_mul(AB[:], x0h[:], wp[:])
            wz_mm(0, out=AB, start=False, stop=True)
            wp = wz_mm(3)
            t3 = ew.tile([P, G], FP16, tag="t3", bufs=1)
            nc.vector.tensor_mul(t3[:], sq0[:], wp[:])
            Av = ew.tile([P, G], FP16, tag="Av", bufs=1)
            nc.vector.tensor_add(Av[:], t3[:], AB[:])
            ya = ew.tile([P, G], FP16, tag="ya", bufs=1)
            nc.vector.tensor_mul(ya[:], x0h[:], Av[:])

            # B-block: BB(psum) = x0*wz4, += wz2 (PE), yb = n1*BB
            wp = wz_mm(4)
            BB = acc_pool.tile([P, G], FP32, tag="zb")
            nc.vector.tensor_mul(BB[:], x0h[:], wp[:])
            wz_mm(2, out=BB, start=False, stop=True)
            yb = ew.tile([P, G], FP16, tag="yb", bufs=1)
            nc.vector.tensor_mul(yb[:], n1h[:], BB[:])
            y0 = ew.tile([P, G], FP16, tag="y0")
            nc.vector.tensor_add(y0[:], ya[:], yb[:])

            # c1-block: CB(psum) = x0*wz6, += wz5 (PE),
            # c1 = CB + sq0*wz7 (+ n1*wz8 on gpsimd)
            wp = wz_mm(6)
            CB = acc_pool.tile([P, G], FP32, tag="zb")
            nc.vector.tensor_mul(CB[:], x0h[:], wp[:])
            wz_mm(5, out=CB, start=False, stop=True)
            wp = wz_mm(7)
            t7 = ew.tile([P, G], FP16, tag="t7", bufs=1)
            nc.vector.tensor_mul(t7[:], sq0[:], wp[:])
            wp = wz_mm(8)
            t8 = ew.tile([P, G], FP16, tag="t8", bufs=1)
            nc.vector.tensor_mul(t8[:], n1h[:], wp[:])
            c1 = ew.tile([P, G], FP16, tag="c1")
            nc.vector.tensor_add(c1[:], t7[:], CB[:])
            nc.gpsimd.tensor_add(c1[:], c1[:], t8[:])

            # y1t = c1*x1 ; smul = y1t*A1 ; sv = sum_m smul
            y1t = ew.tile([P, 3, G], FP16, tag="y1t")
            nc.vector.tensor_mul(y1t[:], bcast3(c1[:]), xh[:])
            smul = ew.tile([P, 3, G], FP16, tag="smul", bufs=1)
            nc.vector.tensor_mul(smul[:], y1t[:], A1[:])
            sv = ew.tile([P, G], FP16, tag="sv")
            nc.gpsimd.tensor_add(sv[:], smul[:, 0, :], smul[:, 1, :])
            nc.gpsimd.tensor_add(sv[:], sv[:], smul[:, 2, :])

            # ------- magmom MLP (channel-major, Silu on Act) -------
            h = iT
            hw_ = [w1h_rep[64:64 + INV, :], w2h[:], w3h[:]]
            for li in range(3):
                hp = zs()
                nc.tensor.matmul(hp[0:64, :], hw_[li], h)
                hn = ew.tile([64, G], FP16, tag=f"h{li}")
                nc.scalar.activation(hn[:], hp[0:64, :], AF.Silu)
                h = hn[:]
            # a0-scaled copy of h3 feeds the wa/wd matmuls (folds a0/SCL in)
            h3a = ew.tile([64, G], FP16, tag="h3a")
            nc.gpsimd.tensor_mul(h3a[:], h, A0h[:])

            # tpw quarters: wa,wd use h3a (a0-scaled); wb,wc use h
            wp = zs()
            nc.tensor.matmul(wp[:], w4h[:, 0:P], h3a[:])
            mid0a = ew.tile([P, G], FP16, tag="mid0a")
            nc.vector.tensor_mul(mid0a[:], y0[:], wp[:])
            wp = zs()
            nc.tensor.matmul(wp[:], w4h[:, P:2 * P], h)
            g2 = ew.tile([P, G], FP16, tag="g2")
            nc.vector.tensor_mul(g2[:], sv[:], wp[:])
            wp = zs()
            nc.tensor.matmul(wp[:], w4h[:, 2 * P:3 * P], h)
            wcy0 = ew.tile([P, G], FP16, tag="wcy0")
            nc.vector.tensor_mul(wcy0[:], y0[:], wp[:])
            wp = zs()
            nc.tensor.matmul(wp[:], w4h[:, 3 * P:4 * P], h3a[:])
            rc2 = ew.tile([P, G], FP16, tag="rc2")
            nc.vector.tensor_mul(rc2[:], c1[:], wp[:])

            m1c = ew.tile([P, 3, G], FP16, tag="m1c", bufs=1)
            nc.vector.tensor_mul(m1c[:], bcast3(wcy0[:]), A1[:])
            hm = ew.tile([P, 3, G], FP16, tag="hm", bufs=1)
            nc.vector.tensor_mul(hm[:], bcast3(rc2[:]), xh[:])

            # ------- output linears: node-major PSUM via mid-stationary -------
            outv = out_st[:].rearrange("p (q f) -> p q f", q=4)
            scv = sc_st[:].rearrange("p (q f) -> p q f", q=4)

            o0p = out_pool.tile([P, 4, P], FP32, tag="o")
            for q in range(4):
                qs = slice(q * P, (q + 1) * P)
                nc.tensor.matmul(o0p[:, q, :], mid0a[:, qs], Wh[:, 0, :], start=True, stop=False)
                nc.tensor.matmul(o0p[:, q, :], g2[:, qs], Wh[:, 1, :], start=False, stop=False)
                nc.tensor.matmul(o0p[:, q, :], y0[:, qs], Wh[:, 4, :], start=False, stop=True)
            nc.vector.tensor_add(outv[:, :, 0:C], o0p[:], scv[:, :, 0:C])

            for m in range(3):
                o1p = out_pool.tile([P, 4, P], FP32, tag="o")
                for q in range(4):
                    qs = slice(q * P, (q + 1) * P)
                    nc.tensor.matmul(o1p[:, q, :], m1c[:, m, qs], Wh[:, 2, :], start=True, stop=False)
                    nc.tensor.matmul(o1p[:, q, :], hm[:, m, qs], Wh[:, 3, :], start=False, stop=False)
                    nc.tensor.matmul(o1p[:, q, :], y1t[:, m, qs], Wh[:, 5, :], start=False, stop=True)
                ovm = outv[:, :, C:4 * C].rearrange("p q (c j) -> p q c j", j=3)[:, :, :, m]
                svm = scv[:, :, C:4 * C].rearrange("p q (c j) -> p q c j", j=3)[:, :, :, m]
                nc.vector.tensor_add(ovm, o1p[:], svm)

            tgt = scr_r if warmup else out_r[:, s_]
            nc.sync.dma_start(out=tgt, in_=out_st[:].rearrange("p (q x) -> p q x", q=4))

        # sacrificial first supertile absorbs cold-start races.
        emit(0, warmup=True)
        for s_ in range(n_st):
            emit(s_)

    nc.compile()
    return nc


_CACHE = {}


def _get_program(n_tiles):
    if n_tiles not in _CACHE:
        _CACHE[n_tiles] = build_program(n_tiles)
    return _CACHE[n_tiles]


def _in_map_for_core(inputs, c, n_core):
    lo, hi = c * n_core, (c + 1) * n_core
    return {
        "node_feats": np.ascontiguousarray(
            inputs["node_feats"][lo:hi].reshape(n_core, 4 * C)
        ),
        "sc": np.ascontiguousarray(inputs["sc"][lo:hi]),
        "node_attrs": np.ascontiguousarray(inputs["node_attrs"][lo:hi]),
        "magmom_node_inv_feats": np.ascontiguousarray(
            inputs["magmom_node_inv_feats"][lo:hi]
        ),
        "magmom_node_attrs": np.ascontiguousarray(inputs["magmom_node_attrs"][lo:hi]),
        "w_sc0": np.ascontiguousarray(inputs["w_sc0"].reshape(E, 5 * C)),
        "w_sc1": np.ascontiguousarray(inputs["w_sc1"].reshape(E, 4 * C)),
        "w_mlp1": np.asarray(inputs["w_mlp1"]),
        "w_mlp2": np.asarray(inputs["w_mlp2"]),
        "w_mlp3": np.asarray(inputs["w_mlp3"]),
        "w_mlp4": np.asarray(inputs["w_mlp4"]),
        "W_l0": np.asarray(inputs["W_l0"]),
        "W_l1": np.asarray(inputs["W_l1"]),
        "Wo0": np.asarray(inputs["Wo0"]),
        "Wo1": np.asarray(inputs["Wo1"]),
    }


def run_on_hw(inputs, trace=False, trace_cores=None):
    inputs = {k: np.asarray(v, dtype=np.float32) for k, v in inputs.items()}
    n_nodes = inputs["node_feats"].shape[0]
    n_core = n_nodes // N_CORES
    nc = _get_program(n_core // P)
    in_maps = [_in_map_for_core(inputs, c, n_core) for c in range(N_CORES)]
    res = run_bass_kernel_spmd(
        nc, in_maps, core_ids=list(range(N_CORES)), trace=trace,
        trace_cores=trace_cores,
    )
    out = np.concatenate([res.results[c]["out"] for c in range(N_CORES)], axis=0)
    return out.astype(np.float32), res


def kernel(**inputs) -> np.ndarray:
    import os, time

    os.environ.setdefault("NEURON_RT_RESET_CORES", "1")
    try:
        out, _ = run_on_hw(inputs, trace=False)
    except Exception:
        time.sleep(5)
        out, _ = run_on_hw(inputs, trace=False)
    return out


# revision 9
# speedup vs baseline: 26.7851x; 1.1319x over previous
"""Trainium2 Bass kernel for nn_EquivariantProductBasisWithSelfMagmomBlock.

Data-parallel over nodes: 8 NeuronCores x 8192 nodes each.

Channel-major design: per 512-node supertile, PE transposes the node-major
inputs into channel-major [c, n] tiles; elementwise math runs mostly on fp16
[128, 512] tiles; matmuls run fp16 with fp32 PSUM accumulation.

v2 changes vs baseline:
 - attrs/inv/mag concatenated into one [128, 30] tile -> 4 input transposes
   per supertile instead of 24.
 - x1 components transpose into one 3-bank PSUM tile; a single Act copy
   moves all three planes to fp16 SBUF.
 - Act Silu directly from PSUM (replaces sigmoid + DVE mul per MLP layer).
 - wz chain restructured: DVE writes x0*wz products straight into PSUM and
   the companion wz term accumulates on top via a start=False matmul.
 - several SBUF-only fp16 adds/muls offloaded to the idle GpSimd engine.

PSUM budget (8 banks): x1p 3 + zs ring 2 + zb 1 + out ring 2.

Node map inside a core: local node n = s*512 + q*128 + p.
"""

import sys

sys.path.insert(0, "/opt/trn_rl_repo")

from contextlib import ExitStack

import numpy as np

import concourse.bass as bass
import concourse.tile as tile
from concourse import bacc, mybir
from concourse.bass_utils import run_bass_kernel_spmd
from concourse.masks import make_identity

FP32 = mybir.dt.float32
F32R = mybir.dt.float32r
FP16 = mybir.dt.float16
AF = mybir.ActivationFunctionType
OP = mybir.AluOpType

N = 65536
C = 128
E = 10
INV = 16
N_CORES = 8
N_CORE = N // N_CORES  # 8192
P = 128
G = 512  # nodes per supertile
CMB = 80  # padded: attrs@0:10, mag@32:36, inv@64:80 (matmul base-partition rule)

SCL = 16.0  # fp16 range guard: A-tiles carry 1/SCL, W_l* weights carry SCL


def bcast3(ap_2d):
    """[p, n] AP -> [p, 3, n] stride-0 broadcast AP on the middle dim."""
    return bass.AP(
        tensor=ap_2d.tensor, offset=ap_2d.offset,
        ap=[ap_2d.ap[0], [0, 3], ap_2d.ap[1]],
    )


def build_program(n_tiles):
    """Build the per-core SPMD program. n_tiles tiles of 128 nodes each."""
    nc = bacc.Bacc(
        "TRN2", target_bir_lowering=False, debug=False, num_devices=N_CORES
    )
    n_nodes = n_tiles * P
    assert n_tiles % 4 == 0
    n_st = n_tiles // 4

    def din(name, shape):
        return nc.dram_tensor(name, list(shape), FP32, kind="ExternalInput").ap()

    nf_d = din("node_feats", (n_nodes, 4 * C))
    sc_d = din("sc", (n_nodes, 4 * C))
    attrs_d = din("node_attrs", (n_nodes, E))
    inv_d = din("magmom_node_inv_feats", (n_nodes, INV))
    mag_d = din("magmom_node_attrs", (n_nodes, 4))
    wsc0_d = din("w_sc0", (E, 5 * C))
    wsc1_d = din("w_sc1", (E, 4 * C))
    w1_d = din("w_mlp1", (INV, 64))
    w2_d = din("w_mlp2", (64, 64))
    w3_d = din("w_mlp3", (64, 64))
    w4_d = din("w_mlp4", (64, 4 * C))
    wl0_d = din("W_l0", (2 * C, C))
    wl1_d = din("W_l1", (2 * C, C))
    wo0_d = din("Wo0", (C, C))
    wo1_d = din("Wo1", (C, C))
    out_d = nc.dram_tensor("out", [n_nodes, 4 * C], FP32, kind="ExternalOutput").ap()
    scr_d = nc.dram_tensor("warmup_scratch", [G, 4 * C], FP32, kind="Internal").ap()
    scr_r = scr_d.rearrange("(q p) x -> p q x", p=P, q=4)

    # node n = s*512 + q*128 + p
    nf_r = nf_d.rearrange("(s q p) x -> p s q x", p=P, q=4)
    sc_r = sc_d.rearrange("(s q p) x -> p s q x", p=P, q=4)
    out_r = out_d.rearrange("(s q p) x -> p s q x", p=P, q=4)
    attrs_r = attrs_d.rearrange("(s q p) x -> p s q x", p=P, q=4)
    inv_r = inv_d.rearrange("(s q p) x -> p s q x", p=P, q=4)
    mag_r = mag_d.rearrange("(s q p) x -> p s q x", p=P, q=4)

    with tile.TileContext(nc) as tc, ExitStack() as ctx:
        singles = ctx.enter_context(tc.tile_pool(name="singles", bufs=1))
        nat = ctx.enter_context(tc.tile_pool(name="nat", bufs=2))
        ew = ctx.enter_context(tc.tile_pool(name="ew", bufs=2))
        # PSUM pools (8 banks): tp 2 + wz 2 + zb 1 + misc 1 + out 2.
        # Per-stage pools decouple supertiles: s+1's transposes don't wait
        # for s's late-stage ring drain.
        tp_pool = ctx.enter_context(tc.tile_pool(name="tp", bufs=2, space="PSUM"))
        wzp_pool = ctx.enter_context(tc.tile_pool(name="wzp", bufs=2, space="PSUM"))
        acc_pool = ctx.enter_context(tc.tile_pool(name="accp", bufs=1, space="PSUM"))
        misc_pool = ctx.enter_context(tc.tile_pool(name="misc", bufs=1, space="PSUM"))
        out_pool = ctx.enter_context(tc.tile_pool(name="outp", bufs=2, space="PSUM"))

        # ---------------- preloads ----------------
        # identity is produced by gpsimd (Q7); launder it through a DVE copy
        # so PE never consumes a Q7-written tensor.
        ident_g = singles.tile([P, P], FP32)
        make_identity(nc, ident_g[:])
        ident = singles.tile([P, P], FP32)

        # combined attrs|mag|inv per-node table, fp32; slice bases chosen so
        # each transposed block lands at a legal matmul base partition.
        cmb_all = singles.tile([P, n_st, 4, CMB], FP32)
        nc.sync.dma_start(out=cmb_all[:, :, :, 0:E], in_=attrs_r)
        nc.sync.dma_start(out=cmb_all[:, :, :, 32:36], in_=mag_r)
        nc.sync.dma_start(out=cmb_all[:, :, :, 64:64 + INV], in_=inv_r)

        wscf = singles.tile([E, 9 * C], FP32)
        nc.sync.dma_start(out=wscf[:, 0:5 * C], in_=wsc0_d)
        nc.sync.dma_start(out=wscf[:, 5 * C:9 * C], in_=wsc1_d)
        wsc_h = singles.tile([E, 9 * C], FP16)
        nc.vector.tensor_copy(wsc_h[:], wscf[:])

        w1f = singles.tile([INV, 64], FP32)
        nc.sync.dma_start(out=w1f[:], in_=w1_d)
        w2f = singles.tile([64, 64], FP32)
        nc.sync.dma_start(out=w2f[:], in_=w2_d)
        w3f = singles.tile([64, 64], FP32)
        nc.sync.dma_start(out=w3f[:], in_=w3_d)
        w4f = singles.tile([64, 4 * C], FP32)
        nc.sync.dma_start(out=w4f[:], in_=w4_d)
        w2h = singles.tile([64, 64], FP16)
        nc.vector.tensor_copy(w2h[:], w2f[:])
        w3h = singles.tile([64, 64], FP16)
        nc.vector.tensor_copy(w3h[:], w3f[:])
        w4h = singles.tile([64, 4 * C], FP16)
        nc.vector.tensor_copy(w4h[:], w4f[:])
        # laundering copy sits late in the in-order DVE queue
        nc.vector.tensor_copy(ident[:], ident_g[:])

        # output weights: 0=WA0*S 1=WB0*S 2=WA1*S 3=WB1*S 4=Wo0 5=Wo1
        Wf = singles.tile([P, 6, C], FP32)
        nc.sync.dma_start(out=Wf[:, 0, :], in_=wl0_d[0:128, :])
        nc.sync.dma_start(out=Wf[:, 1, :], in_=wl0_d[128:256, :])
        nc.sync.dma_start(out=Wf[:, 2, :], in_=wl1_d[0:128, :])
        nc.sync.dma_start(out=Wf[:, 3, :], in_=wl1_d[128:256, :])
        nc.sync.dma_start(out=Wf[:, 4, :], in_=wo0_d)
        nc.sync.dma_start(out=Wf[:, 5, :], in_=wo1_d)
        Wh = singles.tile([P, 6, C], FP16)
        nc.scalar.activation(Wh[:, 0:4, :], Wf[:, 0:4, :], AF.Copy, scale=SCL)
        nc.scalar.copy(Wh[:, 4:6, :], Wf[:, 4:6, :])

        # broadcast stationaries at base 32 (match magh rows): sel[k] picks
        # mag row 32+k and replicates it over all output partitions.
        sel32 = singles.tile([36, 4, P], FP16)
        ones36 = singles.tile([36, P], FP16)
        nc.vector.memset(ones36[:], 1.0 / SCL)
        # plane m selects mag row 32+m: sel[32+k, m, :] = (1/SCL)*delta(k==m),
        # built as ones * per-partition column e_m taken from the identity.
        for m in range(4):
            nc.vector.tensor_scalar_mul(
                sel32[32:36, m, :], ones36[32:36, :], ident_g[32:36, 32 + m:33 + m]
            )
        # MLP layer-1 stationary replica at base 64 (matches iT rows)
        w1h_rep = singles.tile([64 + INV, 64], FP16)
        nc.vector.tensor_copy(w1h_rep[64:64 + INV, :], w1f[:])

        def emit(s_, warmup=False):
            # ---------------- supertile loads ----------------
            nf_st = nat.tile([P, 16 * C], FP32, tag="nf")
            nc.sync.dma_start(
                out=nf_st[:].rearrange("p (q x) -> p q x", q=4), in_=nf_r[:, s_]
            )
            sc_st = nat.tile([P, 16 * C], FP32, tag="sc")
            nc.sync.dma_start(
                out=sc_st[:].rearrange("p (q x) -> p q x", q=4), in_=sc_r[:, s_]
            )
            out_st = nat.tile([P, 16 * C], FP32, tag="out")

            nfv = nf_st[:].rearrange("p (q c j) -> p q c j", q=4, j=4)

            zs_n = [0]

            def ptile(pool, tag):
                zs_n[0] += 1
                return pool.tile([P, G], FP32, tag=tag, name=f"zs{zs_n[0]}")

            # ------- combined attrs|inv|mag transpose: 4 PE ops -------
            cmbp = ptile(tp_pool, "tp")
            for q in range(4):
                nc.tensor.transpose(
                    cmbp[0:CMB, q * P:(q + 1) * P], cmb_all[:, s_, q, :], ident[:]
                )
            cmbh = ew.tile([CMB, G], FP16, tag="cmbh")
            nc.vector.tensor_copy(cmbh[:], cmbp[0:CMB, :])
            aT = cmbh[0:E, :]
            magh = cmbh[32:36, :]  # rows: a0, a1x, a1y, a1z (base 32)
            # (consumed as matmul moving at base 32 with sel32 stationaries)
            iT = cmbh[64:64 + INV, :]  # base 64

            # ------- x transposes -> PSUM; copies to fp16 SBUF -------
            x0p = ptile(tp_pool, "tp")
            for q in range(4):
                nc.tensor.transpose(
                    x0p[:, q * P:(q + 1) * P], nfv[:, q, :, 0], ident[:]
                )
            x0h = ew.tile([P, G], FP16, tag="x0h")
            nc.vector.tensor_copy(x0h[:], x0p[:])
            xh = ew.tile([P, 3, G], FP16, tag="xh")
            for m in range(3):
                x1p = ptile(tp_pool, "tp")
                for q in range(4):
                    nc.tensor.transpose(
                        x1p[:, q * P:(q + 1) * P], nfv[:, q, :, 1 + m], ident[:]
                    )
                nc.scalar.copy(xh[:, m, :], x1p[:])

            # ------- A broadcasts (PE ones-matmul, carries 1/SCL) -------
            A1 = ew.tile([P, 3, G], FP16, tag="A1")
            for m in range(3):
                bp = ptile(misc_pool, "mi")
                nc.tensor.matmul(bp[:], sel32[32:36, 1 + m, :], magh[:])
                nc.scalar.copy(A1[:, m, :], bp[:])
            bp = ptile(misc_pool, "mi")
            nc.tensor.matmul(bp[0:64, :], sel32[32:36, 0, 0:64], magh[:])
            A0h = ew.tile([64, G], FP16, tag="A0h")
            nc.scalar.copy(A0h[:], bp[0:64, :])

            # ------- magmom MLP (hoisted: only needs cmbh) -------
            h = iT
            hw_ = [w1h_rep[64:64 + INV, :], w2h[:], w3h[:]]
            for li in range(3):
                hp = ptile(misc_pool, "mi")
                nc.tensor.matmul(hp[0:64, :], hw_[li], h)
                hn = ew.tile([64, G], FP16, tag=f"h{li}")
                nc.scalar.activation(hn[:], hp[0:64, :], AF.Silu)
                h = hn[:]

            # ------- squares -------
            sq0 = ew.tile([P, G], FP16, tag="sq0")
            nc.vector.tensor_mul(sq0[:], x0h[:], x0h[:])
            sq1 = ew.tile([P, 3, G], FP16, tag="sq1", bufs=1)
            nc.scalar.activation(sq1[:], xh[:], AF.Square)
            n1h = ew.tile([P, G], FP16, tag="n1")
            nc.gpsimd.tensor_add(n1h[:], sq1[:, 0, :], sq1[:, 1, :])
            nc.gpsimd.tensor_add(n1h[:], n1h[:], sq1[:, 2, :])

            # ------- wz chain -------
            # A = wz0 + x0*wz1 + sq0*wz3 ; B = wz2 + x0*wz4
            # c1 = wz5 + x0*wz6 + sq0*wz7 + n1*wz8 ; y0 = x0*A + n1*B
            def wz_mm(k, out=None, start=True, stop=True):
                if out is None:
                    out = ptile(wzp_pool, "wz")
                nc.tensor.matmul(
                    out[:], wsc_h[:, k * P:(k + 1) * P], aT,
                    start=start, stop=stop, skip_group_check=True,
                )
                return out

            # A-block: AB(psum) = x0*wz1, += wz0 (PE), Av = AB + sq0*wz3
            wp = wz_mm(1)
            AB = acc_pool.tile([P, G], FP32, tag="zb")
            nc.vector.tensor_mul(AB[:], x0h[:], wp[:])
            wz_mm(0, out=AB, start=False, stop=True)
            wp = wz_mm(3)
            t3 = ew.tile([P, G], FP16, tag="t3", bufs=1)
            nc.vector.tensor_mul(t3[:], sq0[:], wp[:])
            Av = ew.tile([P, G], FP16, tag="Av", bufs=1)
            nc.vector.tensor_add(Av[:], t3[:], AB[:])
            ya = ew.tile([P, G], FP16, tag="ya", bufs=1)
            nc.vector.tensor_mul(ya[:], x0h[:], Av[:])

            # B-block: BB(psum) = x0*wz4, += wz2 (PE), yb = n1*BB
            wp = wz_mm(4)
            BB = acc_pool.tile([P, G], FP32, tag="zb")
            nc.vector.tensor_mul(BB[:], x0h[:], wp[:])
            wz_mm(2, out=BB, start=False, stop=True)
            yb = ew.tile([P, G], FP16, tag="yb", bufs=1)
            nc.vector.tensor_mul(yb[:], n1h[:], BB[:])
            y0 = ew.tile([P, G], FP16, tag="y0")
            nc.vector.tensor_add(y0[:], ya[:], yb[:])

            # c1-block: CB(psum) = x0*wz6, += wz5 (PE),
            # c1 = CB + sq0*wz7 (+ n1*wz8 on gpsimd)
            wp = wz_mm(6)
            CB = acc_pool.tile([P, G], FP32, tag="zb")
            nc.vector.tensor_mul(CB[:], x0h[:], wp[:])
            wz_mm(5, out=CB, start=False, stop=True)
            wp = wz_mm(7)
            t7 = ew.tile([P, G], FP16, tag="t7", bufs=1)
            nc.vector.tensor_mul(t7[:], sq0[:], wp[:])
            wp = wz_mm(8)
            t8 = ew.tile([P, G], FP16, tag="t8", bufs=1)
            nc.vector.tensor_mul(t8[:], n1h[:], wp[:])
            c1 = ew.tile([P, G], FP16, tag="c1")
            nc.vector.tensor_add(c1[:], t7[:], CB[:])
            nc.gpsimd.tensor_add(c1[:], c1[:], t8[:])

            # y1t = c1*x1 ; smul = y1t*A1 ; sv = sum_m smul
            y1t = ew.tile([P, 3, G], FP16, tag="y1t")
            nc.vector.tensor_mul(y1t[:], bcast3(c1[:]), xh[:])
            smul = ew.tile([P, 3, G], FP16, tag="smul", bufs=1)
            nc.vector.tensor_mul(smul[:], y1t[:], A1[:])
            sv = ew.tile([P, G], FP16, tag="sv")
            nc.gpsimd.tensor_add(sv[:], smul[:, 0, :], smul[:, 1, :])
            nc.gpsimd.tensor_add(sv[:], sv[:], smul[:, 2, :])

            # a0-scaled copy of h3 feeds the wa/wd matmuls (folds a0/SCL in)
            h3a = ew.tile([64, G], FP16, tag="h3a")
            nc.vector.tensor_mul(h3a[:], h, A0h[:])

            # tpw quarters: wa,wd use h3a (a0-scaled); wb,wc use h
            wp = ptile(misc_pool, "mi")
            nc.tensor.matmul(wp[:], w4h[:, 0:P], h3a[:])
            mid0a = ew.tile([P, G], FP16, tag="mid0a")
            nc.vector.tensor_mul(mid0a[:], y0[:], wp[:])
            wp = ptile(misc_pool, "mi")
            nc.tensor.matmul(wp[:], w4h[:, P:2 * P], h)
            g2 = ew.tile([P, G], FP16, tag="g2")
            nc.vector.tensor_mul(g2[:], sv[:], wp[:])
            wp = ptile(misc_pool, "mi")
            nc.tensor.matmul(wp[:], w4h[:, 2 * P:3 * P], h)
            wcy0 = ew.tile([P, G], FP16, tag="wcy0")
            nc.vector.tensor_mul(wcy0[:], y0[:], wp[:])
            wp = ptile(misc_pool, "mi")
            nc.tensor.matmul(wp[:], w4h[:, 3 * P:4 * P], h3a[:])
            rc2 = ew.tile([P, G], FP16, tag="rc2")
            nc.vector.tensor_mul(rc2[:], c1[:], wp[:])

            m1c = ew.tile([P, 3, G], FP16, tag="m1c", bufs=1)
            nc.vector.tensor_mul(m1c[:], bcast3(wcy0[:]), A1[:])
            hm = ew.tile([P, 3, G], FP16, tag="hm", bufs=1)
            nc.vector.tensor_mul(hm[:], bcast3(rc2[:]), xh[:])

            # ------- output linears: node-major PSUM via mid-stationary -------
            outv = out_st[:].rearrange("p (q f) -> p q f", q=4)
            scv = sc_st[:].rearrange("p (q f) -> p q f", q=4)

            o0p = out_pool.tile([P, 4, P], FP32, tag="o")
            for q in range(4):
                qs = slice(q * P, (q + 1) * P)
                nc.tensor.matmul(o0p[:, q, :], mid0a[:, qs], Wh[:, 0, :], start=True, stop=False)
                nc.tensor.matmul(o0p[:, q, :], g2[:, qs], Wh[:, 1, :], start=False, stop=False)
                nc.tensor.matmul(o0p[:, q, :], y0[:, qs], Wh[:, 4, :], start=False, stop=True)
            nc.vector.tensor_add(outv[:, :, 0:C], o0p[:], scv[:, :, 0:C])

            for m in range(3):
                o1p = out_pool.tile([P, 4, P], FP32, tag="o")
                for q in range(4):
                    qs = slice(q * P, (q + 1) * P)
                    nc.tensor.matmul(o1p[:, q, :], m1c[:, m, qs], Wh[:, 2, :], start=True, stop=False)
                    nc.tensor.matmul(o1p[:, q, :], hm[:, m, qs], Wh[:, 3, :], start=False, stop=False)
                    nc.tensor.matmul(o1p[:, q, :], y1t[:, m, qs], Wh[:, 5, :], start=False, stop=True)
                ovm = outv[:, :, C:4 * C].rearrange("p q (c j) -> p q c j", j=3)[:, :, :, m]
                svm = scv[:, :, C:4 * C].rearrange("p q (c j) -> p q c j", j=3)[:, :, :, m]
                nc.vector.tensor_add(ovm, o1p[:], svm)

            tgt = scr_r if warmup else out_r[:, s_]
            nc.sync.dma_start(out=tgt, in_=out_st[:].rearrange("p (q x) -> p q x", q=4))

        # sacrificial first supertile absorbs cold-start races.
        emit(0, warmup=True)
        for s_ in range(n_st):
            emit(s_)

    nc.compile()
    return nc


_CACHE = {}


def _get_program(n_tiles):
    if n_tiles not in _CACHE:
        _CACHE[n_tiles] = build_program(n_tiles)
    return _CACHE[n_tiles]


def _in_map_for_core(inputs, c, n_core):
    lo, hi = c * n_core, (c + 1) * n_core
    return {
        "node_feats": np.ascontiguousarray(
            inputs["node_feats"][lo:hi].reshape(n_core, 4 * C)
        ),
        "sc": np.ascontiguousarray(inputs["sc"][lo:hi]),
        "node_attrs": np.ascontiguousarray(inputs["node_attrs"][lo:hi]),
        "magmom_node_inv_feats": np.ascontiguousarray(
            inputs["magmom_node_inv_feats"][lo:hi]
        ),
        "magmom_node_attrs": np.ascontiguousarray(inputs["magmom_node_attrs"][lo:hi]),
        "w_sc0": np.ascontiguousarray(inputs["w_sc0"].reshape(E, 5 * C)),
        "w_sc1": np.ascontiguousarray(inputs["w_sc1"].reshape(E, 4 * C)),
        "w_mlp1": np.asarray(inputs["w_mlp1"]),
        "w_mlp2": np.asarray(inputs["w_mlp2"]),
        "w_mlp3": np.asarray(inputs["w_mlp3"]),
        "w_mlp4": np.asarray(inputs["w_mlp4"]),
        "W_l0": np.asarray(inputs["W_l0"]),
        "W_l1": np.asarray(inputs["W_l1"]),
        "Wo0": np.asarray(inputs["Wo0"]),
        "Wo1": np.asarray(inputs["Wo1"]),
    }


def run_on_hw(inputs, trace=False, trace_cores=None):
    inputs = {k: np.asarray(v, dtype=np.float32) for k, v in inputs.items()}
    n_nodes = inputs["node_feats"].shape[0]
    n_core = n_nodes // N_CORES
    nc = _get_program(n_core // P)
    in_maps = [_in_map_for_core(inputs, c, n_core) for c in range(N_CORES)]
    res = run_bass_kernel_spmd(
        nc, in_maps, core_ids=list(range(N_CORES)), trace=trace,
        trace_cores=trace_cores,
    )
    out = np.concatenate([res.results[c]["out"] for c in range(N_CORES)], axis=0)
    return out.astype(np.float32), res


def kernel(**inputs) -> np.ndarray:
    import os, time

    os.environ.setdefault("NEURON_RT_RESET_CORES", "1")
    try:
        out, _ = run_on_hw(inputs, trace=False)
    except Exception:
        time.sleep(5)
        out, _ = run_on_hw(inputs, trace=False)
    return out


# revision 12
# speedup vs baseline: 28.4943x; 1.0638x over previous
"""Trainium2 Bass kernel for nn_EquivariantProductBasisWithSelfMagmomBlock.

Data-parallel over nodes: 8 NeuronCores x 8192 nodes each.

Channel-major design: per 512-node supertile, PE transposes the node-major
inputs into channel-major [c, n] tiles; elementwise math runs mostly on fp16
[128, 512] tiles; matmuls run fp16 with fp32 PSUM accumulation.

v2 changes vs baseline:
 - attrs/inv/mag concatenated into one [128, 30] tile -> 4 input transposes
   per supertile instead of 24.
 - x1 components transpose into one 3-bank PSUM tile; a single Act copy
   moves all three planes to fp16 SBUF.
 - Act Silu directly from PSUM (replaces sigmoid + DVE mul per MLP layer).
 - wz chain restructured: DVE writes x0*wz products straight into PSUM and
   the companion wz term accumulates on top via a start=False matmul.
 - several SBUF-only fp16 adds/muls offloaded to the idle GpSimd engine.

PSUM budget (8 banks): x1p 3 + zs ring 2 + zb 1 + out ring 2.

Node map inside a core: local node n = s*512 + q*128 + p.
"""

import sys

sys.path.insert(0, "/opt/trn_rl_repo")

from contextlib import ExitStack

import numpy as np

import concourse.bass as bass
import concourse.tile as tile
from concourse import bacc, mybir
from concourse.bass_utils import run_bass_kernel_spmd
from concourse.masks import make_identity

FP32 = mybir.dt.float32
F32R = mybir.dt.float32r
FP16 = mybir.dt.float16
AF = mybir.ActivationFunctionType
OP = mybir.AluOpType

N = 65536
C = 128
E = 10
INV = 16
N_CORES = 8
N_CORE = N // N_CORES  # 8192
P = 128
G = 512  # nodes per supertile
CMB = 80  # padded: attrs@0:10, mag@32:36, inv@64:80 (matmul base-partition rule)

SCL = 16.0  # fp16 range guard: A-tiles carry 1/SCL, W_l* weights carry SCL


def bcast3(ap_2d):
    """[p, n] AP -> [p, 3, n] stride-0 broadcast AP on the middle dim."""
    return bass.AP(
        tensor=ap_2d.tensor, offset=ap_2d.offset,
        ap=[ap_2d.ap[0], [0, 3], ap_2d.ap[1]],
    )


def build_program(n_tiles):
    """Build the per-core SPMD program. n_tiles tiles of 128 nodes each."""
    nc = bacc.Bacc(
        "TRN2", target_bir_lowering=False, debug=False, num_devices=N_CORES
    )
    n_nodes = n_tiles * P
    assert n_tiles % 4 == 0
    n_st = n_tiles // 4

    def din(name, shape):
        return nc.dram_tensor(name, list(shape), FP32, kind="ExternalInput").ap()

    nf_d = din("node_feats", (n_nodes, 4 * C))
    sc_d = din("sc", (n_nodes, 4 * C))
    attrs_d = din("node_attrs", (n_nodes, E))
    inv_d = din("magmom_node_inv_feats", (n_nodes, INV))
    mag_d = din("magmom_node_attrs", (n_nodes, 4))
    wsc0_d = din("w_sc0", (E, 5 * C))
    wsc1_d = din("w_sc1", (E, 4 * C))
    w1_d = din("w_mlp1", (INV, 64))
    w2_d = din("w_mlp2", (64, 64))
    w3_d = din("w_mlp3", (64, 64))
    w4_d = din("w_mlp4", (64, 4 * C))
    wl0_d = din("W_l0", (2 * C, C))
    wl1_d = din("W_l1", (2 * C, C))
    wo0_d = din("Wo0", (C, C))
    wo1_d = din("Wo1", (C, C))
    out_d = nc.dram_tensor("out", [n_nodes, 4 * C], FP32, kind="ExternalOutput").ap()
    scr_d = nc.dram_tensor("warmup_scratch", [G, 4 * C], FP32, kind="Internal").ap()
    scr_r = scr_d.rearrange("(q p) x -> p q x", p=P, q=4)

    # node n = s*512 + q*128 + p
    nf_r = nf_d.rearrange("(s q p) x -> p s q x", p=P, q=4)
    sc_r = sc_d.rearrange("(s q p) x -> p s q x", p=P, q=4)
    out_r = out_d.rearrange("(s q p) x -> p s q x", p=P, q=4)
    attrs_r = attrs_d.rearrange("(s q p) x -> p s q x", p=P, q=4)
    inv_r = inv_d.rearrange("(s q p) x -> p s q x", p=P, q=4)
    mag_r = mag_d.rearrange("(s q p) x -> p s q x", p=P, q=4)

    with tile.TileContext(nc) as tc, ExitStack() as ctx:
        singles = ctx.enter_context(tc.tile_pool(name="singles", bufs=1))
        nat = ctx.enter_context(tc.tile_pool(name="nat", bufs=2))
        ew = ctx.enter_context(tc.tile_pool(name="ew", bufs=2))
        # PSUM pools (8 banks): tp 2 + wz 2 + zb 1 + misc 1 + out 2.
        # Per-stage pools decouple supertiles: s+1's transposes don't wait
        # for s's late-stage ring drain.
        tp_pool = ctx.enter_context(tc.tile_pool(name="tp", bufs=2, space="PSUM"))
        wzp_pool = ctx.enter_context(tc.tile_pool(name="wzp", bufs=2, space="PSUM"))
        acc_pool = ctx.enter_context(tc.tile_pool(name="accp", bufs=1, space="PSUM"))
        misc_pool = ctx.enter_context(tc.tile_pool(name="misc", bufs=1, space="PSUM"))
        out_pool = ctx.enter_context(tc.tile_pool(name="outp", bufs=2, space="PSUM"))

        # ---------------- preloads ----------------
        # identity is produced by gpsimd (Q7); launder it through a DVE copy
        # so PE never consumes a Q7-written tensor.
        ident_g = singles.tile([P, P], FP32)
        make_identity(nc, ident_g[:])
        ident = singles.tile([P, P], F32R)

        # combined attrs|mag|inv per-node table, fp32; slice bases chosen so
        # each transposed block lands at a legal matmul base partition.
        cmb_all = singles.tile([P, n_st, 4, CMB], F32R)
        nc.sync.dma_start(out=cmb_all[:, :, :, 0:E], in_=attrs_r.bitcast(F32R))
        nc.sync.dma_start(out=cmb_all[:, :, :, 32:36], in_=mag_r.bitcast(F32R))
        nc.sync.dma_start(out=cmb_all[:, :, :, 64:64 + INV], in_=inv_r.bitcast(F32R))

        wscf = singles.tile([E, 9 * C], FP32)
        nc.sync.dma_start(out=wscf[:, 0:5 * C], in_=wsc0_d)
        nc.sync.dma_start(out=wscf[:, 5 * C:9 * C], in_=wsc1_d)
        wsc_h = singles.tile([E, 9 * C], FP16)
        nc.vector.tensor_copy(wsc_h[:], wscf[:])

        w1f = singles.tile([INV, 64], FP32)
        nc.sync.dma_start(out=w1f[:], in_=w1_d)
        w2f = singles.tile([64, 64], FP32)
        nc.sync.dma_start(out=w2f[:], in_=w2_d)
        w3f = singles.tile([64, 64], FP32)
        nc.sync.dma_start(out=w3f[:], in_=w3_d)
        w4f = singles.tile([64, 4 * C], FP32)
        nc.sync.dma_start(out=w4f[:], in_=w4_d)
        w2h = singles.tile([64, 64], FP16)
        nc.vector.tensor_copy(w2h[:], w2f[:])
        w3h = singles.tile([64, 64], FP16)
        nc.vector.tensor_copy(w3h[:], w3f[:])
        w4h = singles.tile([64, 4 * C], FP16)
        nc.vector.tensor_copy(w4h[:], w4f[:])
        # laundering copy sits late in the in-order DVE queue
        nc.vector.tensor_copy(ident[:], ident_g[:])

        # output weights: 0=WA0*S 1=WB0*S 2=WA1*S 3=WB1*S 4=Wo0 5=Wo1
        Wf = singles.tile([P, 6, C], FP32)
        nc.sync.dma_start(out=Wf[:, 0, :], in_=wl0_d[0:128, :])
        nc.sync.dma_start(out=Wf[:, 1, :], in_=wl0_d[128:256, :])
        nc.sync.dma_start(out=Wf[:, 2, :], in_=wl1_d[0:128, :])
        nc.sync.dma_start(out=Wf[:, 3, :], in_=wl1_d[128:256, :])
        nc.sync.dma_start(out=Wf[:, 4, :], in_=wo0_d)
        nc.sync.dma_start(out=Wf[:, 5, :], in_=wo1_d)
        Wh = singles.tile([P, 6, C], FP16)
        nc.scalar.activation(Wh[:, 0:4, :], Wf[:, 0:4, :], AF.Copy, scale=SCL)
        nc.scalar.copy(Wh[:, 4:6, :], Wf[:, 4:6, :])

        # broadcast stationaries at base 32 (match magh rows): sel[k] picks
        # mag row 32+k and replicates it over all output partitions.
        sel32 = singles.tile([36, 4, P], FP16)
        ones36 = singles.tile([36, P], FP16)
        nc.vector.memset(ones36[:], 1.0 / SCL)
        # plane m selects mag row 32+m: sel[32+k, m, :] = (1/SCL)*delta(k==m),
        # built as ones * per-partition column e_m taken from the identity.
        for m in range(4):
            nc.vector.tensor_scalar_mul(
                sel32[32:36, m, :], ones36[32:36, :], ident_g[32:36, 32 + m:33 + m]
            )
        # MLP layer-1 stationary replica at base 64 (matches iT rows)
        w1h_rep = singles.tile([64 + INV, 64], FP16)
        nc.vector.tensor_copy(w1h_rep[64:64 + INV, :], w1f[:])

        def emit(s_, warmup=False):
            # ---------------- supertile loads ----------------
            nf_st = nat.tile([P, 16 * C], F32R, tag="nf")
            nc.sync.dma_start(
                out=nf_st[:].rearrange("p (q x) -> p q x", q=4),
                in_=nf_r[:, s_].bitcast(F32R),
            )
            sc_st = nat.tile([P, 16 * C], FP32, tag="sc")
            nc.sync.dma_start(
                out=sc_st[:].rearrange("p (q x) -> p q x", q=4), in_=sc_r[:, s_]
            )
            out_st = nat.tile([P, 16 * C], FP32, tag="out")

            nfv = nf_st[:].rearrange("p (q c j) -> p q c j", q=4, j=4)

            zs_n = [0]

            def ptile(pool, tag):
                zs_n[0] += 1
                return pool.tile([P, G], FP32, tag=tag, name=f"zs{zs_n[0]}")

            # ------- combined attrs|inv|mag transpose: 4 PE ops -------
            cmbp = ptile(tp_pool, "tp")
            for q in range(4):
                nc.tensor.matmul(
                    cmbp[0:CMB, q * P:(q + 1) * P],
                    cmb_all[:, s_, q, :], ident[:],
                )
            cmbh = ew.tile([CMB, G], FP16, tag="cmbh")
            nc.vector.tensor_copy(cmbh[:], cmbp[0:CMB, :])
            aT = cmbh[0:E, :]
            magh = cmbh[32:36, :]  # rows: a0, a1x, a1y, a1z (base 32)
            # (consumed as matmul moving at base 32 with sel32 stationaries)
            iT = cmbh[64:64 + INV, :]  # base 64

            # ------- x transposes -> PSUM; copies to fp16 SBUF -------
            x0p = ptile(tp_pool, "tp")
            for q in range(4):
                nc.tensor.matmul(
                    x0p[:, q * P:(q + 1) * P],
                    nfv[:, q, :, 0], ident[:],
                )
            x0h = ew.tile([P, G], FP16, tag="x0h")
            nc.vector.tensor_copy(x0h[:], x0p[:])
            xh = ew.tile([P, 3, G], FP16, tag="xh")
            for m in range(3):
                x1p = ptile(tp_pool, "tp")
                for q in range(4):
                    nc.tensor.matmul(
                        x1p[:, q * P:(q + 1) * P],
                        nfv[:, q, :, 1 + m], ident[:],
                    )
                nc.scalar.copy(xh[:, m, :], x1p[:])

            # ------- A broadcasts (PE ones-matmul, carries 1/SCL) -------
            A1 = ew.tile([P, 3, G], FP16, tag="A1")
            for m in range(3):
                bp = ptile(misc_pool, "mi")
                nc.tensor.matmul(bp[:], sel32[32:36, 1 + m, :], magh[:])
                nc.scalar.copy(A1[:, m, :], bp[:])
            bp = ptile(misc_pool, "mi")
            nc.tensor.matmul(bp[0:64, :], sel32[32:36, 0, 0:64], magh[:])
            A0h = ew.tile([64, G], FP16, tag="A0h")
            nc.scalar.copy(A0h[:], bp[0:64, :])

            # ------- magmom MLP (hoisted: only needs cmbh) -------
            h = iT
            hw_ = [w1h_rep[64:64 + INV, :], w2h[:], w3h[:]]
            for li in range(3):
                hp = ptile(misc_pool, "mi")
                nc.tensor.matmul(hp[0:64, :], hw_[li], h)
                hn = ew.tile([64, G], FP16, tag=f"h{li}")
                nc.scalar.activation(hn[:], hp[0:64, :], AF.Silu)
                h = hn[:]

            # ------- squares -------
            sq0 = ew.tile([P, G], FP16, tag="sq0")
            nc.vector.tensor_mul(sq0[:], x0h[:], x0h[:])
            sq1 = ew.tile([P, 3, G], FP16, tag="sq1", bufs=1)
            nc.scalar.activation(sq1[:], xh[:], AF.Square)
            n1h = ew.tile([P, G], FP16, tag="n1")
            nc.gpsimd.tensor_add(n1h[:], sq1[:, 0, :], sq1[:, 1, :])
            nc.gpsimd.tensor_add(n1h[:], n1h[:], sq1[:, 2, :])

            # ------- wz chain -------
            # A = wz0 + x0*wz1 + sq0*wz3 ; B = wz2 + x0*wz4
            # c1 = wz5 + x0*wz6 + sq0*wz7 + n1*wz8 ; y0 = x0*A + n1*B
            def wz_mm(k, out=None, start=True, stop=True):
                if out is None:
                    out = ptile(wzp_pool, "wz")
                nc.tensor.matmul(
                    out[:], wsc_h[:, k * P:(k + 1) * P], aT,
                    start=start, stop=stop, skip_group_check=True,
                )
                return out

            # A-block: AB(psum) = x0*wz1, += wz0 (PE), Av = AB + sq0*wz3
            wp = wz_mm(1)
            AB = acc_pool.tile([P, G], FP32, tag="zb")
            nc.vector.tensor_mul(AB[:], x0h[:], wp[:])
            wz_mm(0, out=AB, start=False, stop=True)
            wp = wz_mm(3)
            t3 = ew.tile([P, G], FP16, tag="t3", bufs=1)
            nc.vector.tensor_mul(t3[:], sq0[:], wp[:])
            Av = ew.tile([P, G], FP16, tag="Av", bufs=1)
            nc.vector.tensor_add(Av[:], t3[:], AB[:])
            ya = ew.tile([P, G], FP16, tag="ya", bufs=1)
            nc.vector.tensor_mul(ya[:], x0h[:], Av[:])

            # B-block: BB(psum) = x0*wz4, += wz2 (PE), yb = n1*BB
            wp = wz_mm(4)
            BB = acc_pool.tile([P, G], FP32, tag="zb")
            nc.vector.tensor_mul(BB[:], x0h[:], wp[:])
            wz_mm(2, out=BB, start=False, stop=True)
            yb = ew.tile([P, G], FP16, tag="yb", bufs=1)
            nc.vector.tensor_mul(yb[:], n1h[:], BB[:])
            y0 = ew.tile([P, G], FP16, tag="y0")
            nc.vector.tensor_add(y0[:], ya[:], yb[:])

            # c1-block: CB(psum) = x0*wz6, += wz5 (PE),
            # c1 = CB + sq0*wz7 (+ n1*wz8 on gpsimd)
            wp = wz_mm(6)
            CB = acc_pool.tile([P, G], FP32, tag="zb")
            nc.vector.tensor_mul(CB[:], x0h[:], wp[:])
            wz_mm(5, out=CB, start=False, stop=True)
            wp = wz_mm(7)
            t7 = ew.tile([P, G], FP16, tag="t7", bufs=1)
            nc.vector.tensor_mul(t7[:], sq0[:], wp[:])
            wp = wz_mm(8)
            t8 = ew.tile([P, G], FP16, tag="t8", bufs=1)
            nc.vector.tensor_mul(t8[:], n1h[:], wp[:])
            c1 = ew.tile([P, G], FP16, tag="c1")
            nc.vector.tensor_add(c1[:], t7[:], CB[:])
            nc.gpsimd.tensor_add(c1[:], c1[:], t8[:])

            # y1t = c1*x1 ; smul = y1t*A1 ; sv = sum_m smul
            y1t = ew.tile([P, 3, G], FP16, tag="y1t")
            nc.vector.tensor_mul(y1t[:], bcast3(c1[:]), xh[:])
            smul = ew.tile([P, 3, G], FP16, tag="smul", bufs=1)
            nc.vector.tensor_mul(smul[:], y1t[:], A1[:])
            sv = ew.tile([P, G], FP16, tag="sv")
            nc.gpsimd.tensor_add(sv[:], smul[:, 0, :], smul[:, 1, :])
            nc.gpsimd.tensor_add(sv[:], sv[:], smul[:, 2, :])

            # a0-scaled copy of h3 feeds the wa/wd matmuls (folds a0/SCL in)
            h3a = ew.tile([64, G], FP16, tag="h3a")
            nc.vector.tensor_mul(h3a[:], h, A0h[:])

            # tpw quarters: wa,wd use h3a (a0-scaled); wb,wc use h
            wp = ptile(misc_pool, "mi")
            nc.tensor.matmul(wp[:], w4h[:, 0:P], h3a[:])
            mid0a = ew.tile([P, G], FP16, tag="mid0a")
            nc.vector.tensor_mul(mid0a[:], y0[:], wp[:])
            wp = ptile(misc_pool, "mi")
            nc.tensor.matmul(wp[:], w4h[:, P:2 * P], h)
            g2 = ew.tile([P, G], FP16, tag="g2")
            nc.vector.tensor_mul(g2[:], sv[:], wp[:])
            wp = ptile(misc_pool, "mi")
            nc.tensor.matmul(wp[:], w4h[:, 2 * P:3 * P], h)
            wcy0 = ew.tile([P, G], FP16, tag="wcy0")
            nc.vector.tensor_mul(wcy0[:], y0[:], wp[:])
            wp = ptile(misc_pool, "mi")
            nc.tensor.matmul(wp[:], w4h[:, 3 * P:4 * P], h3a[:])
            rc2 = ew.tile([P, G], FP16, tag="rc2")
            nc.vector.tensor_mul(rc2[:], c1[:], wp[:])

            m1c = ew.tile([P, 3, G], FP16, tag="m1c", bufs=1)
            nc.vector.tensor_mul(m1c[:], bcast3(wcy0[:]), A1[:])
            hm = ew.tile([P, 3, G], FP16, tag="hm", bufs=1)
            nc.vector.tensor_mul(hm[:], bcast3(rc2[:]), xh[:])

            # ------- output linears: node-major PSUM via mid-stationary -------
            outv = out_st[:].rearrange("p (q f) -> p q f", q=4)
            scv = sc_st[:].rearrange("p (q f) -> p q f", q=4)

            o0p = out_pool.tile([P, 4, P], FP32, tag="o")
            for q in range(4):
                qs = slice(q * P, (q + 1) * P)
                nc.tensor.matmul(o0p[:, q, :], mid0a[:, qs], Wh[:, 0, :], start=True, stop=False)
                nc.tensor.matmul(o0p[:, q, :], g2[:, qs], Wh[:, 1, :], start=False, stop=False)
                nc.tensor.matmul(o0p[:, q, :], y0[:, qs], Wh[:, 4, :], start=False, stop=True)
            nc.vector.tensor_add(outv[:, :, 0:C], o0p[:], scv[:, :, 0:C])

            for m in range(3):
                o1p = out_pool.tile([P, 4, P], FP32, tag="o")
                for q in range(4):
                    qs = slice(q * P, (q + 1) * P)
                    nc.tensor.matmul(o1p[:, q, :], m1c[:, m, qs], Wh[:, 2, :], start=True, stop=False)
                    nc.tensor.matmul(o1p[:, q, :], hm[:, m, qs], Wh[:, 3, :], start=False, stop=False)
                    nc.tensor.matmul(o1p[:, q, :], y1t[:, m, qs], Wh[:, 5, :], start=False, stop=True)
                ovm = outv[:, :, C:4 * C].rearrange("p q (c j) -> p q c j", j=3)[:, :, :, m]
                svm = scv[:, :, C:4 * C].rearrange("p q (c j) -> p q c j", j=3)[:, :, :, m]
                nc.vector.tensor_add(ovm, o1p[:], svm)

            tgt = scr_r if warmup else out_r[:, s_]
            nc.sync.dma_start(out=tgt, in_=out_st[:].rearrange("p (q x) -> p q x", q=4))

        # sacrificial first supertile absorbs cold-start races.
        emit(0, warmup=True)
        for s_ in range(n_st):
            emit(s_)

    nc.compile()
    return nc


_CACHE = {}


def _get_program(n_tiles):
    if n_tiles not in _CACHE:
        _CACHE[n_tiles] = build_program(n_tiles)
    return _CACHE[n_tiles]


def _in_map_for_core(inputs, c, n_core):
    lo, hi = c * n_core, (c + 1) * n_core
    return {
        "node_feats": np.ascontiguousarray(
            inputs["node_feats"][lo:hi].reshape(n_core, 4 * C)
        ),
        "sc": np.ascontiguousarray(inputs["sc"][lo:hi]),
        "node_attrs": np.ascontiguousarray(inputs["node_attrs"][lo:hi]),
        "magmom_node_inv_feats": np.ascontiguousarray(
            inputs["magmom_node_inv_feats"][lo:hi]
        ),
        "magmom_node_attrs": np.ascontiguousarray(inputs["magmom_node_attrs"][lo:hi]),
        "w_sc0": np.ascontiguousarray(inputs["w_sc0"].reshape(E, 5 * C)),
        "w_sc1": np.ascontiguousarray(inputs["w_sc1"].reshape(E, 4 * C)),
        "w_mlp1": np.asarray(inputs["w_mlp1"]),
        "w_mlp2": np.asarray(inputs["w_mlp2"]),
        "w_mlp3": np.asarray(inputs["w_mlp3"]),
        "w_mlp4": np.asarray(inputs["w_mlp4"]),
        "W_l0": np.asarray(inputs["W_l0"]),
        "W_l1": np.asarray(inputs["W_l1"]),
        "Wo0": np.asarray(inputs["Wo0"]),
        "Wo1": np.asarray(inputs["Wo1"]),
    }


def run_on_hw(inputs, trace=False, trace_cores=None):
    inputs = {k: np.asarray(v, dtype=np.float32) for k, v in inputs.items()}
    n_nodes = inputs["node_feats"].shape[0]
    n_core = n_nodes // N_CORES
    nc = _get_program(n_core // P)
    in_maps = [_in_map_for_core(inputs, c, n_core) for c in range(N_CORES)]
    res = run_bass_kernel_spmd(
        nc, in_maps, core_ids=list(range(N_CORES)), trace=trace,
        trace_cores=trace_cores,
    )
    out = np.concatenate([res.results[c]["out"] for c in range(N_CORES)], axis=0)
    return out.astype(np.float32), res


def kernel(**inputs) -> np.ndarray:
    import os, time

    os.environ.setdefault("NEURON_RT_RESET_CORES", "1")
    try:
        out, _ = run_on_hw(inputs, trace=False)
    except Exception:
        time.sleep(5)
        out, _ = run_on_hw(inputs, trace=False)
    return out
